# revision 1
# baseline (speedup 1.0000x reference)
"""LocalIsing energy kernel for Trainium2 (8 NeuronCores, data-parallel over batch).

reference:  energy[b] = x[b] @ J1 + sum_c J2[c] * x[b, p0[c]] * x[b, p1[c]]

The pair term is a quadratic form: scatter-add J2 into W[512,512] at (p0,p1)
(host-side, cheap: 130816 elements), then
    energy[b] = sum_j x[b,j] * (x @ W)[b,j]  +  sum_j x[b,j] * J1[j]
Each core handles 128 batch rows: a [128,512] @ [512,512] fp32 matmul on the
PE (4 accumulating K-tiles) plus two fused multiply+reduce DVE instructions.

Input packing (host side) keeps every device instruction to <=1 sync wait
(PE Matmult supports only one):
  wxt [4,128,640]: per K-tile, W rows (512) || x^T rows (128)  -> one DMA
  xj  [128,1024]:  x shard (512) || J1 broadcast (512)         -> one DMA
"""

import numpy as np
from contextlib import ExitStack

import concourse.tile as tile
from concourse import bacc, mybir
from concourse.bass_utils import run_bass_kernel_spmd

N = 512          # spins
B = 1024         # batch
NCORES = 8
BS = B // NCORES  # 128 rows per core = one partition tile
KT = N // 128     # 4 contraction tiles

_cached_nc = None


def _build():
    f32 = mybir.dt.float32
    nc = bacc.Bacc(
        "TRN2", target_bir_lowering=False, debug=False, num_devices=1
    )
    wxt = nc.dram_tensor("wxt", [KT, 128, N + BS], f32, kind="ExternalInput")
    xj = nc.dram_tensor("xj", [BS, 2 * N], f32, kind="ExternalInput")
    en = nc.dram_tensor("energy", [BS, 1], f32, kind="ExternalOutput")

    wxt_r = wxt.rearrange("k p n -> p k n")

    with tile.TileContext(nc) as tc, ExitStack() as ctx:
        sb = ctx.enter_context(tc.tile_pool(name="sb", bufs=1))
        ps = ctx.enter_context(tc.tile_pool(name="ps", bufs=1, space="PSUM"))

        wxt_sb = sb.tile([128, KT, N + BS], f32)
        nc.sync.dma_start(wxt_sb[:, :, :], wxt_r[:, :, :])
        xj_sb = sb.tile([128, 2 * N], f32)
        nc.sync.dma_start(xj_sb, xj[:, :])

        # e1[b] = sum_j x[b,j] * J1[j]
        scr1 = sb.tile([128, N], f32)
        e1 = sb.tile([128, 1], f32)
        nc.vector.tensor_mul(scr1, xj_sb[:, :N], xj_sb[:, N:])
        nc.vector.tensor_reduce(
            e1, scr1, axis=mybir.AxisListType.X, op=mybir.AluOpType.add
        )

        # y = x @ W   (4 accumulating K-tiles on the PE)
        y = ps.tile([128, N], f32)
        for k in range(KT):
            nc.tensor.matmul(
                y,
                wxt_sb[:, k, N:],      # lhsT = x^T K-tile [128, 128]
                wxt_sb[:, k, :N],      # rhs  = W  K-tile [128, 512]
                start=(k == 0),
                stop=(k == KT - 1),
            )

        # e2[b] = sum_j y[b,j] * x[b,j] ; e = e1 + e2
        scr2 = sb.tile([128, N], f32)
        e2 = sb.tile([128, 1], f32)
        nc.vector.tensor_mul(scr2, y, xj_sb[:, :N])
        nc.vector.tensor_reduce(
            e2, scr2, axis=mybir.AxisListType.X, op=mybir.AluOpType.add
        )
        e = sb.tile([128, 1], f32)
        nc.vector.tensor_add(e, e1, e2)
        nc.sync.dma_start(en[:, :], e)
    nc.finalize()
    return nc


def _pack_inputs(x, J1, J2, pairs):
    x = np.asarray(x, dtype=np.float32)
    J1 = np.asarray(J1, dtype=np.float32)
    J2f = np.asarray(J2, dtype=np.float64)
    pairs = np.asarray(pairs)

    # Scatter-add J2 into W (handles duplicate pairs exactly like the
    # reference's gather-sum).
    idx = pairs[:, 0].astype(np.int64) * N + pairs[:, 1].astype(np.int64)
    W = np.bincount(idx, weights=J2f, minlength=N * N).astype(np.float32)
    W = W.reshape(KT, 128, N)

    in_maps = []
    for c in range(NCORES):
        shard = x[c * BS : (c + 1) * BS]
        wxt = np.concatenate([W, shard.T.reshape(KT, 128, BS)], axis=2)
        xj = np.concatenate([shard, np.broadcast_to(J1, (BS, N))], axis=1)
        in_maps.append(
            {"wxt": np.ascontiguousarray(wxt), "xj": np.ascontiguousarray(xj)}
        )
    return in_maps


def kernel(x, J1, J2, pairs):
    global _cached_nc
    if _cached_nc is None:
        _cached_nc = _build()
    in_maps = _pack_inputs(x, J1, J2, pairs)
    res = run_bass_kernel_spmd(_cached_nc, in_maps, core_ids=list(range(NCORES)))
    return np.concatenate([r["energy"].reshape(-1) for r in res.results])



# revision 14
# speedup vs baseline: 1.6893x; 1.6893x over previous
"""LocalIsing energy kernel for Trainium2 (8 NeuronCores, data-parallel over batch).

reference:  energy[b] = x[b] @ J1 + sum_c J2[c] * x[b, p0[c]] * x[b, p1[c]]

The pair term is a quadratic form: scatter-add J2 into W[512,512] at (p0,p1)
(host-side, cheap: 130816 elements), then
    energy[b] = sum_j x[b,j] * (x @ W)[b,j]  +  x[b] @ J1
Each core handles 128 batch rows.

v2 layout (all bf16 inputs, ~770KB/core vs 1.84MB fp32 baseline):
  mega1 [128, 1536]: per partition p:  xT k-tiles (4x128) || W rows k0 || k1
  mega2 [128, 1536]: W rows k2 || k3 || x shard (512)
  jo    [1, 640]:    J1 (512) || ones (128)
Two DMAs (sync + scalar HWDGE) pipeline k0/k1 matmuls against the second half.
J1 enters the PSUM accumulation as a rank-1 matmul (ones x J1) instead of a
256KB broadcast.  The energy column [128,1] is PE-transposed to [1,128] so the
output DMA is a single 512B packet (the [128,1] layout costs 128 4-byte
packets whose completion semaphore takes ~9us).
"""

import numpy as np
from contextlib import ExitStack

import concourse.tile as tile
from concourse import bacc, mybir
from concourse.bass_utils import run_bass_kernel_spmd
from concourse.masks import make_identity

N = 512          # spins
B = 1024         # batch
NCORES = 8
BS = B // NCORES  # 128 rows per core = one partition tile
KT = N // 128     # 4 contraction tiles
HALF = 1536       # columns per mega half (bf16)

_cached_nc = None


def _build():
    f32 = mybir.dt.float32
    bf16 = mybir.dt.bfloat16
    nc = bacc.Bacc(
        "TRN2", target_bir_lowering=False, debug=False, num_devices=1
    )
    mega1 = nc.dram_tensor("mega1", [BS, HALF], bf16, kind="ExternalInput")
    mega2 = nc.dram_tensor("mega2", [BS, HALF], bf16, kind="ExternalInput")
    jo = nc.dram_tensor("jo", [1, N + BS], bf16, kind="ExternalInput")
    en = nc.dram_tensor("energy", [1, BS], f32, kind="ExternalOutput")

    with tile.TileContext(nc) as tc, ExitStack() as ctx:
        sb = ctx.enter_context(tc.tile_pool(name="sb", bufs=1))
        ps = ctx.enter_context(tc.tile_pool(name="ps", bufs=1, space="PSUM"))

        jo_sb = sb.tile([1, N + BS], bf16)
        m1 = sb.tile([BS, HALF], bf16)
        m2 = sb.tile([BS, HALF], bf16)
        nc.sync.dma_start(jo_sb, jo[:, :])
        nc.sync.dma_start(m1, mega1[:, :])
        nc.scalar.dma_start(m2, mega2[:, :])

        ident = sb.tile([128, 128], f32)
        make_identity(nc, ident)

        # y[b,j] = J1[j] + sum_k x[b,k] W[k,j]   (5 accumulating matmuls)
        y = ps.tile([BS, N], f32)
        nc.tensor.matmul(
            y, jo_sb[:1, N:], jo_sb[:1, :N], start=True, stop=False
        )
        # xT k-tiles all live in m1[:, :512]; W k0/k1 in m1, k2/k3 in m2
        w_tiles = [
            m1[:, N : 2 * N], m1[:, 2 * N : 3 * N],
            m2[:, :N], m2[:, N : 2 * N],
        ]
        for k in range(KT):
            nc.tensor.matmul(
                y,
                m1[:, k * 128 : (k + 1) * 128],
                w_tiles[k],
                start=False,
                stop=(k == KT - 1),
            )

        # e[b] = sum_j y[b,j] * x[b,j]
        xs = m2[:, 2 * N : 3 * N]
        e_col = sb.tile([BS, 1], f32)
        scr = sb.tile([BS, N], f32)
        nc.vector.tensor_mul(scr, y, xs)
        nc.vector.tensor_reduce(
            e_col, scr, axis=mybir.AxisListType.X, op=mybir.AluOpType.add
        )

        # [128,1] -> [1,128] so the output DMA is one 512B packet
        et_ps = ps.tile([1, BS], f32)
        nc.tensor.transpose(et_ps, e_col, ident)
        et_sb = sb.tile([1, BS], f32)
        nc.any.tensor_copy(et_sb, et_ps)
        nc.sync.dma_start(en[:, :], et_sb)
    nc.finalize()
    return nc


def _pack_inputs(x, J1, J2, pairs):
    x = np.asarray(x, dtype=np.float32)
    J1 = np.asarray(J1, dtype=np.float32)
    J2f = np.asarray(J2, dtype=np.float64)
    pairs = np.asarray(pairs)
    bf16 = mybir.dt.np(mybir.dt.bfloat16)

    # Scatter-add J2 into W (handles duplicate pairs exactly like the
    # reference's gather-sum).
    idx = pairs[:, 0].astype(np.int64) * N + pairs[:, 1].astype(np.int64)
    W = np.bincount(idx, weights=J2f, minlength=N * N).astype(np.float32)
    # Wp[p, k, :] = W[k*128+p, :]
    Wp = np.ascontiguousarray(
        W.reshape(KT, 128, N).transpose(1, 0, 2).reshape(128, KT * N)
    ).astype(bf16)
    jo = np.concatenate([J1, np.ones(BS, np.float32)]).astype(bf16)[None, :]

    in_maps = []
    for c in range(NCORES):
        xs = x[c * BS : (c + 1) * BS]                      # [128, 512]
        # xtp[p, k*128+b] = x[c*128+b, k*128+p]
        xtp = np.ascontiguousarray(
            xs.T.reshape(KT, 128, BS).transpose(1, 0, 2).reshape(128, KT * BS)
        ).astype(bf16)
        mega1 = np.concatenate([xtp, Wp[:, :N], Wp[:, N : 2 * N]], axis=1)
        mega2 = np.concatenate(
            [Wp[:, 2 * N : 3 * N], Wp[:, 3 * N :], xs.astype(bf16)], axis=1
        )
        in_maps.append(
            {
                "mega1": np.ascontiguousarray(mega1),
                "mega2": np.ascontiguousarray(mega2),
                "jo": jo,
            }
        )
    return in_maps


def kernel(x, J1, J2, pairs):
    global _cached_nc
    if _cached_nc is None:
        _cached_nc = _build()
    in_maps = _pack_inputs(x, J1, J2, pairs)
    res = run_bass_kernel_spmd(_cached_nc, in_maps, core_ids=list(range(NCORES)))
    return np.concatenate([r["energy"].reshape(-1) for r in res.results])


# revision 15
# speedup vs baseline: 1.8378x; 1.0879x over previous
"""LocalIsing energy kernel for Trainium2 (8 NeuronCores, data-parallel over batch).

reference:  energy[b] = x[b] @ J1 + sum_c J2[c] * x[b, p0[c]] * x[b, p1[c]]

The pair term is a quadratic form: scatter-add J2 into W[512,512] at (p0,p1)
(host-side, cheap: 130816 elements), then
    energy[b] = sum_j x[b,j] * (x @ W)[b,j]  +  x[b] @ J1
Each core handles 128 batch rows.

Layout (all bf16 inputs, ~770KB/core vs 1.84MB fp32 baseline):
  mega1 [128, 1536]: per partition p:  xT k-tiles (4x128) || W rows k0 || k1
  mega2 [128, 1536]: W rows k2 || k3 || x shard (512)
  jo    [1, 640]:    J1 (512) || ones (128)
Both mega halves go through the scalar-engine HWDGE queues (the sync-engine
queues move the same bytes ~2.3x slower); jo + nothing else rides sync, issued
in parallel.  J1 enters the PSUM accumulation as a rank-1 matmul (ones x J1)
instead of a 256KB broadcast.  The energy column [128,1] is repacked via the
DVE 32x32 block transpose so the output DMA is 4 x 128B packets instead of
128 x 4B packets (whose completion semaphore costs ~9us).
"""

import numpy as np
from contextlib import ExitStack

import concourse.tile as tile
from concourse import bacc, mybir
from concourse.bass_utils import run_bass_kernel_spmd

N = 512          # spins
B = 1024         # batch
NCORES = 8
BS = B // NCORES  # 128 rows per core = one partition tile
KT = N // 128     # 4 contraction tiles
HALF = 1536       # columns per mega half (bf16)

_cached_nc = None


def _build():
    f32 = mybir.dt.float32
    bf16 = mybir.dt.bfloat16
    nc = bacc.Bacc(
        "TRN2", target_bir_lowering=False, debug=False, num_devices=1
    )
    mega1 = nc.dram_tensor("mega1", [BS, HALF], bf16, kind="ExternalInput")
    mega2 = nc.dram_tensor("mega2", [BS, HALF], bf16, kind="ExternalInput")
    jo = nc.dram_tensor("jo", [1, N + BS], bf16, kind="ExternalInput")
    en = nc.dram_tensor("energy", [4, 32], f32, kind="ExternalOutput")

    with tile.TileContext(nc) as tc, ExitStack() as ctx:
        sb = ctx.enter_context(tc.tile_pool(name="sb", bufs=1))
        ps = ctx.enter_context(tc.tile_pool(name="ps", bufs=1, space="PSUM"))

        jo_sb = sb.tile([1, N + BS], bf16)
        m1 = sb.tile([BS, HALF], bf16)
        m2 = sb.tile([BS, HALF], bf16)
        nc.scalar.dma_start(m1, mega1[:, :])
        nc.scalar.dma_start(m2, mega2[:, :])
        nc.sync.dma_start(jo_sb, jo[:, :])

        # staging block for the 32x32 transpose; column 0 = energy
        ecol = sb.tile([BS, 32], f32)
        nc.gpsimd.memset(ecol, 0.0)

        # y[b,j] = J1[j] + sum_k x[b,k] W[k,j]   (5 accumulating matmuls)
        y = ps.tile([BS, N], f32)
        nc.tensor.matmul(
            y, jo_sb[:1, N:], jo_sb[:1, :N], start=True, stop=False
        )
        # xT k-tiles all live in m1[:, :512]; W k0/k1 in m1, k2/k3 in m2
        w_tiles = [
            m1[:, N : 2 * N], m1[:, 2 * N : 3 * N],
            m2[:, :N], m2[:, N : 2 * N],
        ]
        for k in range(KT):
            nc.tensor.matmul(
                y,
                m1[:, k * 128 : (k + 1) * 128],
                w_tiles[k],
                start=False,
                stop=(k == KT - 1),
            )

        # e[b] = sum_j y[b,j] * x[b,j]
        xs = m2[:, 2 * N : 3 * N]
        scr = sb.tile([BS, N], f32)
        nc.vector.tensor_mul(scr, y, xs)
        nc.vector.tensor_reduce(
            ecol[:, 0:1], scr, axis=mybir.AxisListType.X, op=mybir.AluOpType.add
        )

        # 32x32 block transpose: row 32*g of `et` holds e[32g : 32g+32],
        # so the output DMA is 4 contiguous 128B packets instead of 128x4B.
        et = sb.tile([BS, 32], f32)
        nc.vector.transpose(et, ecol)
        nc.scalar.dma_start(en[:, :], et[0:BS:32, :])
    nc.finalize()
    return nc


def _pack_inputs(x, J1, J2, pairs):
    x = np.asarray(x, dtype=np.float32)
    J1 = np.asarray(J1, dtype=np.float32)
    J2f = np.asarray(J2, dtype=np.float64)
    pairs = np.asarray(pairs)
    bf16 = mybir.dt.np(mybir.dt.bfloat16)

    # Scatter-add J2 into W (handles duplicate pairs exactly like the
    # reference's gather-sum).
    idx = pairs[:, 0].astype(np.int64) * N + pairs[:, 1].astype(np.int64)
    W = np.bincount(idx, weights=J2f, minlength=N * N).astype(np.float32)
    # Wp[p, k, :] = W[k*128+p, :]
    Wp = np.ascontiguousarray(
        W.reshape(KT, 128, N).transpose(1, 0, 2).reshape(128, KT * N)
    ).astype(bf16)
    jo = np.concatenate([J1, np.ones(BS, np.float32)]).astype(bf16)[None, :]

    in_maps = []
    for c in range(NCORES):
        xs = x[c * BS : (c + 1) * BS]                      # [128, 512]
        # xtp[p, k*128+b] = x[c*128+b, k*128+p]
        xtp = np.ascontiguousarray(
            xs.T.reshape(KT, 128, BS).transpose(1, 0, 2).reshape(128, KT * BS)
        ).astype(bf16)
        mega1 = np.concatenate([xtp, Wp[:, :N], Wp[:, N : 2 * N]], axis=1)
        mega2 = np.concatenate(
            [Wp[:, 2 * N : 3 * N], Wp[:, 3 * N :], xs.astype(bf16)], axis=1
        )
        in_maps.append(
            {
                "mega1": np.ascontiguousarray(mega1),
                "mega2": np.ascontiguousarray(mega2),
                "jo": jo,
            }
        )
    return in_maps


def kernel(x, J1, J2, pairs):
    global _cached_nc
    if _cached_nc is None:
        _cached_nc = _build()
    in_maps = _pack_inputs(x, J1, J2, pairs)
    res = run_bass_kernel_spmd(_cached_nc, in_maps, core_ids=list(range(NCORES)))
    return np.concatenate([r["energy"].reshape(-1) for r in res.results])


# revision 24
# speedup vs baseline: 1.9297x; 1.0500x over previous
"""LocalIsing energy kernel for Trainium2 (8 NeuronCores, data-parallel over batch).

reference:  energy[b] = x[b] @ J1 + sum_c J2[c] * x[b, p0[c]] * x[b, p1[c]]

The pair term is a quadratic form: scatter-add J2 into W[512,512] at (p0,p1)
(host-side, cheap: 130816 elements), then
    energy[b] = sum_j x[b,j] * (x @ W)[b,j]  +  x[b] @ J1
Each core handles 128 batch rows.

Since x_i*W_ij*x_j is symmetric in (i,j), any lower-triangle mass is folded
into the upper triangle host-side (Wu = triu(W+W.T,1) + diag(W)), so W is
strictly upper-triangular: K-tile k only has nonzero columns [128k, 512).
Packing just those ranges cuts the W stream from 512KB to 320KB bf16 and the
PE moving-operand columns from 2048 to 1280.

Streams (bf16, ~577KB/core vs 1.84MB fp32 baseline):
  mega1 [128, 1024]: xt k-tiles (4x128) || w0 (512)      scalar HWDGE
  mega1b[128,  640]: w1 (384) || w2 (256)                scalar HWDGE
  mega2 [128,  640]: w3 (128) || xs (512)                sync HWDGE
  jo    [1, 640]:    J1 (512) || ones (128)              sync HWDGE
Each matmul is gated only on the DMA chunk it needs, so the PE chases the
incoming stream.  J1 enters the PSUM accumulation as a rank-1 matmul
(ones x J1) instead of a 256KB broadcast.  The energy column [128,1] is
repacked via the DVE 32x32 block transpose so the output DMA is 4 x 128B
packets instead of 128 x 4B packets (whose completion costs ~9us).
"""

import numpy as np
from contextlib import ExitStack

import concourse.tile as tile
from concourse import bacc, mybir
from concourse.bass_utils import run_bass_kernel_spmd

N = 512          # spins
B = 1024         # batch
NCORES = 8
BS = B // NCORES  # 128 rows per core = one partition tile
KT = N // 128     # 4 contraction tiles
CA = 1024         # mega1 cols:  xt(512) | w0(512)
CB = 640          # mega1b cols: w1(384) | w2(256)
CC = 640          # mega2 cols:  w3(128) | xs(512)

_cached_nc = None


def _build():
    f32 = mybir.dt.float32
    bf16 = mybir.dt.bfloat16
    nc = bacc.Bacc(
        "TRN2", target_bir_lowering=False, debug=False, num_devices=1
    )
    mega1 = nc.dram_tensor("mega1", [BS, CA], bf16, kind="ExternalInput")
    mega1b = nc.dram_tensor("mega1b", [BS, CB], bf16, kind="ExternalInput")
    mega2 = nc.dram_tensor("mega2", [BS, CC], bf16, kind="ExternalInput")
    jo = nc.dram_tensor("jo", [1, N + BS], bf16, kind="ExternalInput")
    en = nc.dram_tensor("energy", [4, 32], f32, kind="ExternalOutput")

    with tile.TileContext(nc) as tc, ExitStack() as ctx:
        sb = ctx.enter_context(tc.tile_pool(name="sb", bufs=1))
        ps = ctx.enter_context(tc.tile_pool(name="ps", bufs=1, space="PSUM"))

        jo_sb = sb.tile([1, N + BS], bf16)
        m1a = sb.tile([BS, CA], bf16)
        m1b = sb.tile([BS, CB], bf16)
        m2 = sb.tile([BS, CC], bf16)
        nc.scalar.dma_start(m1a, mega1[:, :])
        nc.scalar.dma_start(m1b, mega1b[:, :])
        nc.sync.dma_start(m2, mega2[:, :])
        nc.sync.dma_start(jo_sb, jo[:, :])

        # staging block for the 32x32 transpose; column 0 = energy
        ecol = sb.tile([BS, 32], f32)
        nc.gpsimd.memset(ecol, 0.0)

        # y[b,j] = J1[j] + sum_k x[b,k] W[k,j]   (5 accumulating matmuls;
        # tile k of the strictly-upper-tri W only writes y[:, 128k:])
        y = ps.tile([BS, N], f32)
        nc.tensor.matmul(
            y, jo_sb[:1, N:], jo_sb[:1, :N], start=True, stop=False
        )
        w_tiles = [
            (m1a[:, N:], 0),          # w0: cols [0, 512)
            (m1b[:, : 3 * 128], 128), # w1: cols [128, 512)
            (m1b[:, 3 * 128 :], 256), # w2: cols [256, 512)
            (m2[:, :128], 384),       # w3: cols [384, 512)
        ]
        for k, (w, c0) in enumerate(w_tiles):
            nc.tensor.matmul(
                y[:, c0:],
                m1a[:, k * 128 : (k + 1) * 128],
                w,
                start=False,
                stop=(k == KT - 1),
            )

        # e[b] = sum_j y[b,j] * x[b,j]  (fused multiply + row-sum on DVE)
        xs = m2[:, 128:]
        scr = sb.tile([BS, N], f32)
        nc.vector.scalar_tensor_tensor(
            out=scr,
            in0=y,
            scalar=1.0,
            in1=xs,
            op0=mybir.AluOpType.bypass,
            op1=mybir.AluOpType.mult,
            accum_out=ecol[:, 0:1],
        )

        # 32x32 block transpose: row 32*g of `et` holds e[32g : 32g+32],
        # so the output DMA is 4 contiguous 128B packets instead of 128x4B.
        et = sb.tile([BS, 32], f32)
        nc.vector.transpose(et, ecol)
        nc.scalar.dma_start(en[:, :], et[0:BS:32, :])
    nc.finalize()
    return nc


def _pack_inputs(x, J1, J2, pairs):
    x = np.asarray(x, dtype=np.float32)
    J1 = np.asarray(J1, dtype=np.float32)
    J2f = np.asarray(J2, dtype=np.float64)
    pairs = np.asarray(pairs)
    bf16 = mybir.dt.np(mybir.dt.bfloat16)

    # Scatter-add J2 into W (handles duplicate pairs exactly like the
    # reference's gather-sum), then fold the (symmetric) quadratic form
    # into a strictly-upper-triangular matrix.
    idx = pairs[:, 0].astype(np.int64) * N + pairs[:, 1].astype(np.int64)
    W = np.bincount(idx, weights=J2f, minlength=N * N).reshape(N, N)
    Wu = (np.triu(W + W.T, 1) + np.diag(np.diag(W))).astype(np.float32)
    # w_k[p, :] = Wu[128k + p, 128k:]  (the nonzero columns of row 128k+p)
    wk = [
        np.ascontiguousarray(Wu[k * 128 : (k + 1) * 128, k * 128 :]).astype(bf16)
        for k in range(KT)
    ]
    jo = np.concatenate([J1, np.ones(BS, np.float32)]).astype(bf16)[None, :]

    in_maps = []
    for c in range(NCORES):
        xs = x[c * BS : (c + 1) * BS]                      # [128, 512]
        # xtp[p, k*128+b] = x[c*128+b, k*128+p]
        xtp = np.ascontiguousarray(
            xs.T.reshape(KT, 128, BS).transpose(1, 0, 2).reshape(128, KT * BS)
        ).astype(bf16)
        in_maps.append(
            {
                "mega1": np.ascontiguousarray(np.concatenate([xtp, wk[0]], axis=1)),
                "mega1b": np.ascontiguousarray(np.concatenate([wk[1], wk[2]], axis=1)),
                "mega2": np.ascontiguousarray(
                    np.concatenate([wk[3], xs.astype(bf16)], axis=1)
                ),
                "jo": jo,
            }
        )
    return in_maps


def kernel(x, J1, J2, pairs):
    global _cached_nc
    if _cached_nc is None:
        _cached_nc = _build()
    in_maps = _pack_inputs(x, J1, J2, pairs)
    res = run_bass_kernel_spmd(_cached_nc, in_maps, core_ids=list(range(NCORES)))
    return np.concatenate([r["energy"].reshape(-1) for r in res.results])
